# revision 7
# baseline (speedup 1.0000x reference)
"""Trainium2 Bass kernel for nn_DebugBertLayer_87093346828840.

Key observation: the reference overwrites q/k/v with the constant 0.01, so
softmax(scores) is uniform and ctx == 0.01 everywhere.  Hence
    attn_out = LN1(hidden + cvec),   cvec = 0.01 * Wo.sum(axis=1) + bo
and the only real device work is the FFN:
    out = LN2( gelu(attn_out @ Wi.T + bi) @ Wf.T + bf + attn_out )

Sharding: pure data-parallel over the 8192 tokens -> 1024 tokens/core on 8
NeuronCores, no collectives.  Matmuls run in bf16 (fp32 PE matmul is 4x
slower), fp32 PSUM accumulation, fp32 LN/residual path.

Per-core dataflow (token tile = 128 tokens, two token-halves of 512):
  1. LN1 in token-major layout ((x+cvec), bn_stats over the 768 features);
     bf16 copy of the result on the Scalar engine.
  2. PE-transpose the bf16 LN output to feature-major aT [768, 512] per half.
  3. mm1 per half: hT[ff, tok] accumulated over 6 k-tiles (WiT stationary),
     gelu(+bi per-partition bias) psum -> bf16 hT in SBUF.  Phase A (tokens
     0:512) starts after only half the LN1 work; phase B's LN1/transposes
     hide under phase A's matmuls.
  4. mm2: y[tok, 768] accumulated over 24 ff-tiles with hT slices stationary,
     WfT streaming -> token-major psum.
  5. y + a (residual, fp32) + bf, LN2, DMA out.  No output-side transpose
     because mm2's stationary operand is the feature-major hT.
"""

import os
import sys

for _p in ("/opt/trn_rl_repo", "/root/.axon_site/_ro/trn_rl_repo"):
    if os.path.isdir(_p) and _p not in sys.path:
        sys.path.insert(0, _p)

import numpy as np
import ml_dtypes

import concourse.bass as bass
import concourse.bacc as bacc
import concourse.tile as tile
from concourse import mybir
from concourse.bass_utils import run_bass_kernel_spmd

F32 = mybir.dt.float32
BF16 = mybir.dt.bfloat16
AF = mybir.ActivationFunctionType
ALU = mybir.AluOpType
BF16NP = ml_dtypes.bfloat16

D = 768           # d_model
FF = 3072         # d_ff
NCORE = 8
TOK = 8192        # total tokens (4 x 2048)
TPC = TOK // NCORE  # 1024 tokens per core
KD = D // 128     # 6 k-tiles over d_model
MF = FF // 128    # 24 tiles over d_ff
NT = TPC // 128   # 8 token tiles per core
NTH = NT // 2     # token tiles per half
HALF = TPC // 2   # 512
LN_EPS = 1e-12

_NC_CACHE = {}
LAST_RESULTS = None
RUN_KWARGS = {}


def _ln_tile(nc, pstat, eps_t, s_t, gb, apply_gb):
    """In-place LayerNorm over the free dim (768) of s_t [128, 768] f32."""
    g_b, b_b = gb
    sr = s_t.rearrange("p (n s) -> p n s", s=256)
    stats = pstat.tile([128, 3, 6], F32, tag="stats")
    for i in range(3):
        nc.vector.bn_stats(out=stats[:, i, :], in_=sr[:, i, :])
    mv = pstat.tile([128, 2], F32, tag="mv")
    nc.vector.bn_aggr(out=mv[:], in_=stats[:])
    nc.scalar.activation(out=mv[:, 1:2], in_=mv[:, 1:2], func=AF.Sqrt,
                         bias=eps_t[:], scale=1.0)
    nc.vector.reciprocal(out=mv[:, 1:2], in_=mv[:, 1:2])
    nc.vector.tensor_scalar(out=s_t[:], in0=s_t[:], scalar1=mv[:, 0:1],
                            scalar2=mv[:, 1:2], op0=ALU.subtract, op1=ALU.mult)
    if apply_gb:
        nc.vector.tensor_mul(out=s_t[:], in0=s_t[:], in1=g_b[:])
        nc.vector.tensor_add(out=s_t[:], in0=s_t[:], in1=b_b[:])


def _emit_body(nc, tc, pools, tensors, flags, x_pre=None):
    """Emit one full layer computation (one 'rep').

    x_pre: optionally pre-loaded x tiles (first rep: their DMAs were issued
    ahead of the weight DMAs so LN1 isn't stuck behind 9MB of weights).
    """
    (pw, px, pbig, pabf, pstat, pout, ps1, psm) = pools
    (x, y, wi_tiles, wf_tiles, ident_sb, cvec_b, g1_b, b1_b, g2_b, b2_b,
     bfv_b, bi_sb, eps_t) = tensors

    a_tiles = [None] * NT
    aT_half = [None, None]

    def ln1_and_transpose(half):
        aT = pbig.tile([128, KD * HALF], BF16, tag=f"aT{half}")
        aT_half[half] = aT
        for tt in range(NTH):
            t = half * NTH + tt
            if x_pre is not None:
                x_t = x_pre[t]
            else:
                x_t = px.tile([128, D], F32, tag="xa")
                nc.sync.dma_start(out=x_t[:], in_=x[t * 128:(t + 1) * 128, :])
            nc.vector.tensor_add(out=x_t[:], in0=x_t[:], in1=cvec_b[:])
            _ln_tile(nc, pstat, eps_t, x_t, (g1_b, b1_b), flags["g1b1"])
            a_tiles[t] = x_t
            a_bf = pabf.tile([128, D], BF16, tag="abf")
            nc.vector.tensor_copy(out=a_bf[:], in_=x_t[:])
            for k in range(KD):
                ps_tr = psm.tile([128, 128], BF16, tag="psm")
                nc.tensor.transpose(ps_tr[:], a_bf[:, k * 128:(k + 1) * 128],
                                    ident_sb[:])
                nc.vector.tensor_copy(
                    out=aT[:, k * HALF + tt * 128: k * HALF + (tt + 1) * 128],
                    in_=ps_tr[:])

    def mm1_phase(half, hT):
        aT = aT_half[half]
        for m in range(MF):
            ps_a = ps1.tile([128, 512], F32, tag="hps")
            for k in range(KD):
                lhsT = wi_tiles[k][:, m * 128:(m + 1) * 128]
                nc.tensor.matmul(ps_a[:], lhsT, aT[:, k * HALF:(k + 1) * HALF],
                                 start=(k == 0), stop=(k == KD - 1))
            if flags["bi"]:
                nc.scalar.activation(
                    out=hT[:, m * TPC + half * HALF: m * TPC + (half + 1) * HALF],
                    in_=ps_a[:], func=AF.Gelu, bias=bi_sb[:, m:m + 1], scale=1.0)
            else:
                nc.scalar.activation(
                    out=hT[:, m * TPC + half * HALF: m * TPC + (half + 1) * HALF],
                    in_=ps_a[:], func=AF.Gelu, scale=1.0)

    # ---- LN1 + transpose (first half), mm1 phase A, second half, phase B ----
    hT = pbig.tile([128, MF * TPC], BF16, tag="hT")   # [ff-part, m*1024 + tok]
    ln1_and_transpose(0)
    mm1_phase(0, hT)
    ln1_and_transpose(1)
    mm1_phase(1, hT)

    # ---------------- mm2 + residual + LN2 ----------------
    for t in range(NT):
        ps2 = psm.tile([128, D], F32, tag="psm")
        for m in range(MF):
            lhsT = hT[:, m * TPC + t * 128: m * TPC + (t + 1) * 128]
            nc.tensor.matmul(ps2[:, 0:512], lhsT, wf_tiles[m][:, 0:512],
                             start=(m == 0), stop=(m == MF - 1))
            nc.tensor.matmul(ps2[:, 512:768], lhsT, wf_tiles[m][:, 512:768],
                             start=(m == 0), stop=(m == MF - 1))
        s_t = pout.tile([128, D], F32, tag="s")
        nc.vector.tensor_add(out=s_t[:], in0=ps2[:], in1=a_tiles[t][:])
        if flags["bfv"]:
            nc.vector.tensor_add(out=s_t[:], in0=s_t[:], in1=bfv_b[:])
        _ln_tile(nc, pstat, eps_t, s_t, (g2_b, b2_b), flags["g2b2"])
        nc.sync.dma_start(out=y[t * 128:(t + 1) * 128, :], in_=s_t[:])


def _bcast_ap(handle, n):
    """AP that broadcasts a [n]-vector across 128 partitions for DMA."""
    return bass.AP(tensor=handle, offset=0, ap=[[0, 128], [1, n]])


def _build(n_reps=1, flag_key=(True, True, True, True)):
    cache_key = (n_reps, flag_key)
    if cache_key in _NC_CACHE:
        return _NC_CACHE[cache_key]
    flags = dict(zip(("g1b1", "g2b2", "bi", "bfv"), flag_key))
    nc = bacc.Bacc("TRN2", target_bir_lowering=False, debug=False,
                   num_devices=NCORE)
    x = nc.dram_tensor("x", [TPC, D], F32, kind="ExternalInput")
    wi = nc.dram_tensor("wi", [KD, 128, FF], BF16, kind="ExternalInput")
    wf = nc.dram_tensor("wf", [MF, 128, D], BF16, kind="ExternalInput")
    cvec = nc.dram_tensor("cvec", [D], F32, kind="ExternalInput")
    g1 = nc.dram_tensor("g1", [D], F32, kind="ExternalInput")
    b1 = nc.dram_tensor("b1", [D], F32, kind="ExternalInput")
    g2 = nc.dram_tensor("g2", [D], F32, kind="ExternalInput")
    b2 = nc.dram_tensor("b2", [D], F32, kind="ExternalInput")
    bfv = nc.dram_tensor("bfv", [D], F32, kind="ExternalInput")
    bi = nc.dram_tensor("bi", [FF], F32, kind="ExternalInput")
    y = nc.dram_tensor("y", [TPC, D], F32, kind="ExternalOutput")
    ident = nc.inline_tensor(np.eye(128, dtype=BF16NP), name="ident")

    with tile.TileContext(nc) as tc:
        with (
            tc.tile_pool(name="pw", bufs=1) as pw,
            tc.tile_pool(name="px", bufs=NT) as px,
            tc.tile_pool(name="pbig", bufs=1) as pbig,
            tc.tile_pool(name="pabf", bufs=2) as pabf,
            tc.tile_pool(name="pstat", bufs=4) as pstat,
            tc.tile_pool(name="pout", bufs=3) as pout,
            tc.tile_pool(name="ps1", bufs=4, space="PSUM") as ps1,
            tc.tile_pool(name="psm", bufs=2, space="PSUM") as psm,
        ):
            # small/early DMAs first: identity, broadcast constants
            ident_sb = pw.tile([128, 128], BF16, tag="ident")
            nc.sync.dma_start(out=ident_sb[:], in_=ident.ap())

            def bcast(handle, n, tag):
                t = pw.tile([128, n], F32, tag=tag)
                nc.gpsimd.dma_start(out=t[:], in_=_bcast_ap(handle, n))
                return t

            cvec_b = bcast(cvec, D, "cvec")
            g1_b = bcast(g1, D, "g1") if flags["g1b1"] else None
            b1_b = bcast(b1, D, "b1") if flags["g1b1"] else None
            g2_b = bcast(g2, D, "g2") if flags["g2b2"] else None
            b2_b = bcast(b2, D, "b2") if flags["g2b2"] else None
            bfv_b = bcast(bfv, D, "bfv") if flags["bfv"] else None
            bi_sb = None
            if flags["bi"]:
                # bi as [128, 24]: column m holds bi[m*128 : (m+1)*128]
                bi_sb = pw.tile([128, MF], F32, tag="bi")
                nc.gpsimd.dma_start(
                    out=bi_sb[:],
                    in_=bass.AP(tensor=bi, offset=0, ap=[[1, 128], [128, MF]]))
            eps_t = pw.tile([128, 1], F32, tag="eps")
            nc.vector.memset(eps_t[:], LN_EPS)

            # DMA order = HBM arrival order: first-half x tiles, then the
            # mm1 weights, then second-half x, then the mm2 weights.
            x_pre = []
            for t in range(NTH):
                x_t = px.tile([128, D], F32, tag="xa")
                nc.sync.dma_start(out=x_t[:], in_=x[t * 128:(t + 1) * 128, :])
                x_pre.append(x_t)
            wi_tiles = []
            for k in range(KD):
                wt = pw.tile([128, FF], BF16, tag=f"wi{k}")
                nc.sync.dma_start(out=wt[:], in_=wi[k])
                wi_tiles.append(wt)
            for t in range(NTH, NT):
                x_t = px.tile([128, D], F32, tag="xa")
                nc.sync.dma_start(out=x_t[:], in_=x[t * 128:(t + 1) * 128, :])
                x_pre.append(x_t)
            wf_tiles = []
            for m in range(MF):
                wt = pw.tile([128, D], BF16, tag=f"wf{m}")
                nc.sync.dma_start(out=wt[:], in_=wf[m])
                wf_tiles.append(wt)

            tensors = (x, y, wi_tiles, wf_tiles, ident_sb, cvec_b, g1_b, b1_b,
                       g2_b, b2_b, bfv_b, bi_sb, eps_t)
            pools = (pw, px, pbig, pabf, pstat, pout, ps1, psm)
            if isinstance(n_reps, tuple):  # ("loop", n) -> dynamic Tile loop
                with tc.For_i(0, n_reps[1], 1):
                    _emit_body(nc, tc, pools, tensors, flags)
            else:
                for i in range(n_reps):
                    _emit_body(nc, tc, pools, tensors, flags,
                               x_pre=x_pre if i == 0 else None)

    nc.compile()
    _NC_CACHE[cache_key] = nc
    return nc


def _prep_inputs(hidden_states, Wo, bo, ln1_g, ln1_b, Wi, bi, Wf, bf,
                 ln2_g, ln2_b):
    x = np.ascontiguousarray(np.asarray(hidden_states, np.float32)
                             .reshape(TOK, D))
    Wo = np.asarray(Wo, np.float32)
    Wi = np.asarray(Wi, np.float32)
    Wf = np.asarray(Wf, np.float32)
    cvec = (0.01 * Wo.sum(axis=1) + np.asarray(bo, np.float32)).astype(np.float32)
    # wi layout: [k, p, f] = Wi.T[k*128+p, f]
    wi_prep = np.ascontiguousarray(
        Wi.T.reshape(KD, 128, FF).astype(BF16NP))
    # wf layout: [m, p, j] = Wf.T[m*128+p, j]
    wf_prep = np.ascontiguousarray(
        Wf.T.reshape(MF, 128, D).astype(BF16NP))
    common = {
        "wi": wi_prep, "wf": wf_prep, "cvec": cvec,
        "g1": np.asarray(ln1_g, np.float32), "b1": np.asarray(ln1_b, np.float32),
        "g2": np.asarray(ln2_g, np.float32), "b2": np.asarray(ln2_b, np.float32),
        "bfv": np.asarray(bf, np.float32), "bi": np.asarray(bi, np.float32),
    }
    in_maps = [dict(common, x=x[c * TPC:(c + 1) * TPC]) for c in range(NCORE)]
    flag_key = (
        not (np.all(ln1_g == 1.0) and np.all(ln1_b == 0.0)),
        not (np.all(ln2_g == 1.0) and np.all(ln2_b == 0.0)),
        bool(np.any(np.asarray(bi) != 0.0)),
        bool(np.any(np.asarray(bf) != 0.0)),
    )
    return in_maps, flag_key


def kernel(hidden_states, Wq, bq, Wk, bk, Wv, bv, Wo, bo, ln1_g, ln1_b,
           Wi, bi, Wf, bf, ln2_g, ln2_b):
    global LAST_RESULTS
    B, S, _ = hidden_states.shape
    in_maps, flag_key = _prep_inputs(hidden_states, Wo, bo, ln1_g, ln1_b,
                                     Wi, bi, Wf, bf, ln2_g, ln2_b)
    nc = _build(RUN_KWARGS.get("n_reps", 1), flag_key)
    res = run_bass_kernel_spmd(nc, in_maps, list(range(NCORE)),
                               **{k: v for k, v in RUN_KWARGS.items()
                                  if k != "n_reps"})
    LAST_RESULTS = res
    out = np.concatenate([res.results[c]["y"] for c in range(NCORE)], axis=0)
    return np.ascontiguousarray(out.reshape(B, S, D).astype(np.float32))


# revision 8
# speedup vs baseline: 1.0642x; 1.0642x over previous
"""Trainium2 Bass kernel for nn_DebugBertLayer_87093346828840.

Key observation: the reference overwrites q/k/v with the constant 0.01, so
softmax(scores) is uniform and ctx == 0.01 everywhere.  Hence
    attn_out = LN1(hidden + cvec),   cvec = 0.01 * Wo.sum(axis=1) + bo
and the only real device work is the FFN:
    out = LN2( gelu(attn_out @ Wi.T + bi) @ Wf.T + bf + attn_out )

Sharding: pure data-parallel over the 8192 tokens -> 1024 tokens/core on 8
NeuronCores, no collectives.  Matmuls run in bf16 (fp32 PE matmul is 4x
slower), fp32 PSUM accumulation, fp32 LN/residual path.

Per-core dataflow (token tile = 128 tokens, two token-halves of 512):
  1. LN1 in token-major layout ((x+cvec), bn_stats over the 768 features);
     bf16 copy of the result on the Scalar engine.
  2. PE-transpose the bf16 LN output to feature-major aT [768, 512] per half.
  3. mm1 per half: hT[ff, tok] accumulated over 6 k-tiles (WiT stationary),
     gelu(+bi per-partition bias) psum -> bf16 hT in SBUF.  Phase A (tokens
     0:512) starts after only half the LN1 work; phase B's LN1/transposes
     hide under phase A's matmuls.
  4. mm2: y[tok, 768] accumulated over 24 ff-tiles with hT slices stationary,
     WfT streaming -> token-major psum.
  5. y + a (residual, fp32) + bf, LN2, DMA out.  No output-side transpose
     because mm2's stationary operand is the feature-major hT.
"""

import os
import sys

for _p in ("/opt/trn_rl_repo", "/root/.axon_site/_ro/trn_rl_repo"):
    if os.path.isdir(_p) and _p not in sys.path:
        sys.path.insert(0, _p)

import numpy as np
import ml_dtypes

import concourse.bass as bass
import concourse.bacc as bacc
import concourse.tile as tile
from concourse import mybir
from concourse.bass_utils import run_bass_kernel_spmd

F32 = mybir.dt.float32
BF16 = mybir.dt.bfloat16
AF = mybir.ActivationFunctionType
ALU = mybir.AluOpType
BF16NP = ml_dtypes.bfloat16

D = 768           # d_model
FF = 3072         # d_ff
NCORE = 8
TOK = 8192        # total tokens (4 x 2048)
TPC = TOK // NCORE  # 1024 tokens per core
KD = D // 128     # 6 k-tiles over d_model
MF = FF // 128    # 24 tiles over d_ff
NT = TPC // 128   # 8 token tiles per core
NTH = NT // 2     # token tiles per half
HALF = TPC // 2   # 512
LN_EPS = 1e-12

_NC_CACHE = {}
LAST_RESULTS = None
RUN_KWARGS = {}


def _ln_tile(nc, pstat, eps_t, s_t, gb, apply_gb):
    """In-place LayerNorm over the free dim (768) of s_t [128, 768] f32."""
    g_b, b_b = gb
    sr = s_t.rearrange("p (n s) -> p n s", s=256)
    stats = pstat.tile([128, 3, 6], F32, tag="stats")
    for i in range(3):
        nc.vector.bn_stats(out=stats[:, i, :], in_=sr[:, i, :])
    mv = pstat.tile([128, 2], F32, tag="mv")
    nc.vector.bn_aggr(out=mv[:], in_=stats[:])
    nc.scalar.activation(out=mv[:, 1:2], in_=mv[:, 1:2], func=AF.Sqrt,
                         bias=eps_t[:], scale=1.0)
    nc.vector.reciprocal(out=mv[:, 1:2], in_=mv[:, 1:2])
    nc.vector.tensor_scalar(out=s_t[:], in0=s_t[:], scalar1=mv[:, 0:1],
                            scalar2=mv[:, 1:2], op0=ALU.subtract, op1=ALU.mult)
    if apply_gb:
        nc.vector.tensor_mul(out=s_t[:], in0=s_t[:], in1=g_b[:])
        nc.vector.tensor_add(out=s_t[:], in0=s_t[:], in1=b_b[:])


def _emit_body(nc, tc, pools, tensors, flags, x_pre=None):
    """Emit one full layer computation (one 'rep').

    x_pre: optionally pre-loaded x tiles (first rep: their DMAs were issued
    ahead of the weight DMAs so LN1 isn't stuck behind 9MB of weights).
    """
    (pw, px, pbig, pabf, pstat, pout, ps1, psm) = pools
    (x, y, wi_tiles, wf_tiles, ident_sb, cvec_b, g1_b, b1_b, g2_b, b2_b,
     bfv_b, bi_sb, eps_t) = tensors

    a_tiles = [None] * NT
    aT_half = [None, None]

    def ln1_and_transpose(half):
        aT = pbig.tile([128, KD * HALF], BF16, tag=f"aT{half}")
        aT_half[half] = aT
        for tt in range(NTH):
            t = half * NTH + tt
            if x_pre is not None:
                x_t = x_pre[t]
            else:
                x_t = px.tile([128, D], F32, tag="xa")
                nc.sync.dma_start(out=x_t[:], in_=x[t * 128:(t + 1) * 128, :])
            nc.vector.tensor_add(out=x_t[:], in0=x_t[:], in1=cvec_b[:])
            _ln_tile(nc, pstat, eps_t, x_t, (g1_b, b1_b), flags["g1b1"])
            a_tiles[t] = x_t
            a_bf = pabf.tile([128, D], BF16, tag="abf")
            nc.vector.tensor_copy(out=a_bf[:], in_=x_t[:])
            for k in range(KD):
                ps_tr = psm.tile([128, 128], BF16, tag="psm")
                nc.tensor.transpose(ps_tr[:], a_bf[:, k * 128:(k + 1) * 128],
                                    ident_sb[:])
                nc.vector.tensor_copy(
                    out=aT[:, k * HALF + tt * 128: k * HALF + (tt + 1) * 128],
                    in_=ps_tr[:])

    def mm1_phase(half, hT):
        aT = aT_half[half]
        for m in range(MF):
            ps_a = ps1.tile([128, 512], F32, tag="hps")
            for k in range(KD):
                lhsT = wi_tiles[k][:, m * 128:(m + 1) * 128]
                nc.tensor.matmul(ps_a[:], lhsT, aT[:, k * HALF:(k + 1) * HALF],
                                 start=(k == 0), stop=(k == KD - 1))
            if flags["bi"]:
                nc.scalar.activation(
                    out=hT[:, m * TPC + half * HALF: m * TPC + (half + 1) * HALF],
                    in_=ps_a[:], func=AF.Gelu, bias=bi_sb[:, m:m + 1], scale=1.0)
            else:
                nc.scalar.activation(
                    out=hT[:, m * TPC + half * HALF: m * TPC + (half + 1) * HALF],
                    in_=ps_a[:], func=AF.Gelu, scale=1.0)

    # ---- LN1 + transpose (first half), mm1 phase A, second half, phase B ----
    hT = pbig.tile([128, MF * TPC], BF16, tag="hT")   # [ff-part, m*1024 + tok]
    ln1_and_transpose(0)
    mm1_phase(0, hT)
    ln1_and_transpose(1)
    mm1_phase(1, hT)

    # ---------------- mm2 + residual + LN2 ----------------
    for t in range(NT):
        ps2 = psm.tile([128, D], F32, tag="psm")
        for m in range(MF):
            lhsT = hT[:, m * TPC + t * 128: m * TPC + (t + 1) * 128]
            nc.tensor.matmul(ps2[:, 0:512], lhsT, wf_tiles[m][:, 0:512],
                             start=(m == 0), stop=(m == MF - 1))
            nc.tensor.matmul(ps2[:, 512:768], lhsT, wf_tiles[m][:, 512:768],
                             start=(m == 0), stop=(m == MF - 1))
        s_t = pout.tile([128, D], F32, tag="s")
        nc.vector.tensor_add(out=s_t[:], in0=ps2[:], in1=a_tiles[t][:])
        if flags["bfv"]:
            nc.vector.tensor_add(out=s_t[:], in0=s_t[:], in1=bfv_b[:])
        _ln_tile(nc, pstat, eps_t, s_t, (g2_b, b2_b), flags["g2b2"])
        nc.sync.dma_start(out=y[t * 128:(t + 1) * 128, :], in_=s_t[:])


def _bcast_ap(handle, n):
    """AP that broadcasts a [n]-vector across 128 partitions for DMA."""
    return bass.AP(tensor=handle, offset=0, ap=[[0, 128], [1, n]])


def _build(n_reps=1, flag_key=(True, True, True, True)):
    cache_key = (n_reps, flag_key)
    if cache_key in _NC_CACHE:
        return _NC_CACHE[cache_key]
    flags = dict(zip(("g1b1", "g2b2", "bi", "bfv"), flag_key))
    nc = bacc.Bacc("TRN2", target_bir_lowering=False, debug=False,
                   num_devices=NCORE)
    x = nc.dram_tensor("x", [TPC, D], F32, kind="ExternalInput")
    wi = nc.dram_tensor("wi", [KD, 128, FF], BF16, kind="ExternalInput")
    wf = nc.dram_tensor("wf", [MF, 128, D], BF16, kind="ExternalInput")
    cvec = nc.dram_tensor("cvec", [D], F32, kind="ExternalInput")
    g1 = nc.dram_tensor("g1", [D], F32, kind="ExternalInput")
    b1 = nc.dram_tensor("b1", [D], F32, kind="ExternalInput")
    g2 = nc.dram_tensor("g2", [D], F32, kind="ExternalInput")
    b2 = nc.dram_tensor("b2", [D], F32, kind="ExternalInput")
    bfv = nc.dram_tensor("bfv", [D], F32, kind="ExternalInput")
    bi = nc.dram_tensor("bi", [FF], F32, kind="ExternalInput")
    y = nc.dram_tensor("y", [TPC, D], F32, kind="ExternalOutput")
    ident = nc.inline_tensor(np.eye(128, dtype=BF16NP), name="ident")

    with tile.TileContext(nc) as tc:
        with (
            tc.tile_pool(name="pw", bufs=1) as pw,
            tc.tile_pool(name="px", bufs=NT) as px,
            tc.tile_pool(name="pbig", bufs=1) as pbig,
            tc.tile_pool(name="pabf", bufs=2) as pabf,
            tc.tile_pool(name="pstat", bufs=4) as pstat,
            tc.tile_pool(name="pout", bufs=3) as pout,
            tc.tile_pool(name="ps1", bufs=3, space="PSUM") as ps1,
            tc.tile_pool(name="psm", bufs=2, space="PSUM") as psm,
        ):
            # small/early DMAs first: identity, broadcast constants
            ident_sb = pw.tile([128, 128], BF16, tag="ident")
            nc.sync.dma_start(out=ident_sb[:], in_=ident.ap())

            def bcast(handle, n, tag):
                t = pw.tile([128, n], F32, tag=tag)
                nc.gpsimd.dma_start(out=t[:], in_=_bcast_ap(handle, n))
                return t

            cvec_b = bcast(cvec, D, "cvec")
            g1_b = bcast(g1, D, "g1") if flags["g1b1"] else None
            b1_b = bcast(b1, D, "b1") if flags["g1b1"] else None
            g2_b = bcast(g2, D, "g2") if flags["g2b2"] else None
            b2_b = bcast(b2, D, "b2") if flags["g2b2"] else None
            bfv_b = bcast(bfv, D, "bfv") if flags["bfv"] else None
            bi_sb = None
            if flags["bi"]:
                # bi as [128, 24]: column m holds bi[m*128 : (m+1)*128]
                bi_sb = pw.tile([128, MF], F32, tag="bi")
                nc.gpsimd.dma_start(
                    out=bi_sb[:],
                    in_=bass.AP(tensor=bi, offset=0, ap=[[1, 128], [128, MF]]))
            eps_t = pw.tile([128, 1], F32, tag="eps")
            nc.vector.memset(eps_t[:], LN_EPS)

            # DMA order = HBM arrival order: first-half x tiles, then the
            # mm1 weights, then second-half x, then the mm2 weights.
            x_pre = []
            for t in range(NTH):
                x_t = px.tile([128, D], F32, tag="xa")
                nc.sync.dma_start(out=x_t[:], in_=x[t * 128:(t + 1) * 128, :])
                x_pre.append(x_t)
            wi_tiles = []
            for k in range(KD):
                wt = pw.tile([128, FF], BF16, tag=f"wi{k}")
                nc.sync.dma_start(out=wt[:], in_=wi[k])
                wi_tiles.append(wt)
            for t in range(NTH, NT):
                x_t = px.tile([128, D], F32, tag="xa")
                nc.sync.dma_start(out=x_t[:], in_=x[t * 128:(t + 1) * 128, :])
                x_pre.append(x_t)
            wf_tiles = []
            for m in range(MF):
                wt = pw.tile([128, D], BF16, tag=f"wf{m}")
                nc.sync.dma_start(out=wt[:], in_=wf[m])
                wf_tiles.append(wt)

            tensors = (x, y, wi_tiles, wf_tiles, ident_sb, cvec_b, g1_b, b1_b,
                       g2_b, b2_b, bfv_b, bi_sb, eps_t)
            pools = (pw, px, pbig, pabf, pstat, pout, ps1, psm)
            if isinstance(n_reps, tuple):  # ("loop", n) -> dynamic Tile loop
                with tc.For_i(0, n_reps[1], 1):
                    _emit_body(nc, tc, pools, tensors, flags)
            else:
                for i in range(n_reps):
                    _emit_body(nc, tc, pools, tensors, flags,
                               x_pre=x_pre if i == 0 else None)

    nc.compile()
    _NC_CACHE[cache_key] = nc
    return nc


def _prep_inputs(hidden_states, Wo, bo, ln1_g, ln1_b, Wi, bi, Wf, bf,
                 ln2_g, ln2_b):
    x = np.ascontiguousarray(np.asarray(hidden_states, np.float32)
                             .reshape(TOK, D))
    Wo = np.asarray(Wo, np.float32)
    Wi = np.asarray(Wi, np.float32)
    Wf = np.asarray(Wf, np.float32)
    cvec = (0.01 * Wo.sum(axis=1) + np.asarray(bo, np.float32)).astype(np.float32)
    # wi layout: [k, p, f] = Wi.T[k*128+p, f]
    wi_prep = np.ascontiguousarray(
        Wi.T.reshape(KD, 128, FF).astype(BF16NP))
    # wf layout: [m, p, j] = Wf.T[m*128+p, j]
    wf_prep = np.ascontiguousarray(
        Wf.T.reshape(MF, 128, D).astype(BF16NP))
    common = {
        "wi": wi_prep, "wf": wf_prep, "cvec": cvec,
        "g1": np.asarray(ln1_g, np.float32), "b1": np.asarray(ln1_b, np.float32),
        "g2": np.asarray(ln2_g, np.float32), "b2": np.asarray(ln2_b, np.float32),
        "bfv": np.asarray(bf, np.float32), "bi": np.asarray(bi, np.float32),
    }
    in_maps = [dict(common, x=x[c * TPC:(c + 1) * TPC]) for c in range(NCORE)]
    flag_key = (
        not (np.all(ln1_g == 1.0) and np.all(ln1_b == 0.0)),
        not (np.all(ln2_g == 1.0) and np.all(ln2_b == 0.0)),
        bool(np.any(np.asarray(bi) != 0.0)),
        bool(np.any(np.asarray(bf) != 0.0)),
    )
    return in_maps, flag_key


def kernel(hidden_states, Wq, bq, Wk, bk, Wv, bv, Wo, bo, ln1_g, ln1_b,
           Wi, bi, Wf, bf, ln2_g, ln2_b):
    global LAST_RESULTS
    B, S, _ = hidden_states.shape
    in_maps, flag_key = _prep_inputs(hidden_states, Wo, bo, ln1_g, ln1_b,
                                     Wi, bi, Wf, bf, ln2_g, ln2_b)
    nc = _build(RUN_KWARGS.get("n_reps", 1), flag_key)
    res = run_bass_kernel_spmd(nc, in_maps, list(range(NCORE)),
                               **{k: v for k, v in RUN_KWARGS.items()
                                  if k != "n_reps"})
    LAST_RESULTS = res
    out = np.concatenate([res.results[c]["y"] for c in range(NCORE)], axis=0)
    return np.ascontiguousarray(out.reshape(B, S, D).astype(np.float32))


# revision 9
# speedup vs baseline: 1.0761x; 1.0112x over previous
"""Trainium2 Bass kernel for nn_DebugBertLayer_87093346828840.

Key observation: the reference overwrites q/k/v with the constant 0.01, so
softmax(scores) is uniform and ctx == 0.01 everywhere.  Hence
    attn_out = LN1(hidden + cvec),   cvec = 0.01 * Wo.sum(axis=1) + bo
and the only real device work is the FFN:
    out = LN2( gelu(attn_out @ Wi.T + bi) @ Wf.T + bf + attn_out )

Sharding: pure data-parallel over the 8192 tokens -> 1024 tokens/core on 8
NeuronCores, no collectives.  Matmuls run in bf16 (fp32 PE matmul is 4x
slower), fp32 PSUM accumulation, fp32 LN/residual path.

Per-core dataflow (token tile = 128 tokens, two token-halves of 512):
  1. LN1 in token-major layout ((x+cvec), bn_stats over the 768 features);
     bf16 copy of the result on the Scalar engine.
  2. PE-transpose the bf16 LN output to feature-major aT [768, 512] per half.
  3. mm1 per half: hT[ff, tok] accumulated over 6 k-tiles (WiT stationary),
     gelu(+bi per-partition bias) psum -> bf16 hT in SBUF.  Phase A (tokens
     0:512) starts after only half the LN1 work; phase B's LN1/transposes
     hide under phase A's matmuls.
  4. mm2: y[tok, 768] accumulated over 24 ff-tiles with hT slices stationary,
     WfT streaming -> token-major psum.
  5. y + a (residual, fp32) + bf, LN2, DMA out.  No output-side transpose
     because mm2's stationary operand is the feature-major hT.
"""

import os
import sys

for _p in ("/opt/trn_rl_repo", "/root/.axon_site/_ro/trn_rl_repo"):
    if os.path.isdir(_p) and _p not in sys.path:
        sys.path.insert(0, _p)

import numpy as np
import ml_dtypes

import concourse.bass as bass
import concourse.bacc as bacc
import concourse.tile as tile
from concourse import mybir
from concourse.bass_utils import run_bass_kernel_spmd

F32 = mybir.dt.float32
BF16 = mybir.dt.bfloat16
AF = mybir.ActivationFunctionType
ALU = mybir.AluOpType
BF16NP = ml_dtypes.bfloat16

D = 768           # d_model
FF = 3072         # d_ff
NCORE = 8
TOK = 8192        # total tokens (4 x 2048)
TPC = TOK // NCORE  # 1024 tokens per core
KD = D // 128     # 6 k-tiles over d_model
MF = FF // 128    # 24 tiles over d_ff
NT = TPC // 128   # 8 token tiles per core
NTH = NT // 2     # token tiles per half
HALF = TPC // 2   # 512
LN_EPS = 1e-12

_NC_CACHE = {}
LAST_RESULTS = None
RUN_KWARGS = {}


def _ln_tile(nc, pstat, eps_t, s_t, gb, apply_gb):
    """In-place LayerNorm over the free dim (768) of s_t [128, 768] f32."""
    g_b, b_b = gb
    sr = s_t.rearrange("p (n s) -> p n s", s=256)
    stats = pstat.tile([128, 3, 6], F32, tag="stats")
    for i in range(3):
        nc.vector.bn_stats(out=stats[:, i, :], in_=sr[:, i, :])
    mv = pstat.tile([128, 2], F32, tag="mv")
    nc.vector.bn_aggr(out=mv[:], in_=stats[:])
    nc.scalar.activation(out=mv[:, 1:2], in_=mv[:, 1:2], func=AF.Sqrt,
                         bias=eps_t[:], scale=1.0)
    nc.vector.reciprocal(out=mv[:, 1:2], in_=mv[:, 1:2])
    nc.vector.tensor_scalar(out=s_t[:], in0=s_t[:], scalar1=mv[:, 0:1],
                            scalar2=mv[:, 1:2], op0=ALU.subtract, op1=ALU.mult)
    if apply_gb:
        nc.vector.tensor_mul(out=s_t[:], in0=s_t[:], in1=g_b[:])
        nc.vector.tensor_add(out=s_t[:], in0=s_t[:], in1=b_b[:])


def _emit_body(nc, tc, pools, tensors, flags, x_pre=None):
    """Emit one full layer computation (one 'rep').

    x_pre: optionally pre-loaded x tiles (first rep: their DMAs were issued
    ahead of the weight DMAs so LN1 isn't stuck behind 9MB of weights).
    """
    (pw, px, pbig, pabf, pstat, pout, ps1, psm) = pools
    (x, y, wi_tiles, wf_tiles, ident_sb, cvec_b, g1_b, b1_b, g2_b, b2_b,
     bfv_b, bi_sb, eps_t) = tensors

    a_tiles = [None] * NT
    aT_half = [None, None]

    def ln1_and_transpose(half):
        aT = pbig.tile([128, KD * HALF], BF16, tag=f"aT{half}")
        aT_half[half] = aT
        for tt in range(NTH):
            t = half * NTH + tt
            if x_pre is not None:
                x_t = x_pre[t]
            else:
                x_t = px.tile([128, D], F32, tag="xa")
                nc.sync.dma_start(out=x_t[:], in_=x[t * 128:(t + 1) * 128, :])
            nc.vector.tensor_add(out=x_t[:], in0=x_t[:], in1=cvec_b[:])
            _ln_tile(nc, pstat, eps_t, x_t, (g1_b, b1_b), flags["g1b1"])
            a_tiles[t] = x_t
            a_bf = pabf.tile([128, D], BF16, tag="abf")
            nc.vector.tensor_copy(out=a_bf[:], in_=x_t[:])
            for k in range(KD):
                nc.sync.dma_start(
                    out=aT[:, k * HALF + tt * 128: k * HALF + (tt + 1) * 128],
                    in_=a_bf[:, k * 128:(k + 1) * 128], transpose=True)

    def mm1_phase(half, hT):
        aT = aT_half[half]
        for m in range(MF):
            ps_a = ps1.tile([128, 512], F32, tag="hps")
            for k in range(KD):
                lhsT = wi_tiles[k][:, m * 128:(m + 1) * 128]
                nc.tensor.matmul(ps_a[:], lhsT, aT[:, k * HALF:(k + 1) * HALF],
                                 start=(k == 0), stop=(k == KD - 1))
            if flags["bi"]:
                nc.scalar.activation(
                    out=hT[:, m * TPC + half * HALF: m * TPC + (half + 1) * HALF],
                    in_=ps_a[:], func=AF.Gelu, bias=bi_sb[:, m:m + 1], scale=1.0)
            else:
                nc.scalar.activation(
                    out=hT[:, m * TPC + half * HALF: m * TPC + (half + 1) * HALF],
                    in_=ps_a[:], func=AF.Gelu, scale=1.0)

    # ---- LN1 + transpose (first half), mm1 phase A, second half, phase B ----
    hT = pbig.tile([128, MF * TPC], BF16, tag="hT")   # [ff-part, m*1024 + tok]
    ln1_and_transpose(0)
    mm1_phase(0, hT)
    ln1_and_transpose(1)
    mm1_phase(1, hT)

    # ---------------- mm2 + residual + LN2 ----------------
    for t in range(NT):
        ps2 = psm.tile([128, D], F32, tag="psm")
        for m in range(MF):
            lhsT = hT[:, m * TPC + t * 128: m * TPC + (t + 1) * 128]
            nc.tensor.matmul(ps2[:, 0:512], lhsT, wf_tiles[m][:, 0:512],
                             start=(m == 0), stop=(m == MF - 1))
            nc.tensor.matmul(ps2[:, 512:768], lhsT, wf_tiles[m][:, 512:768],
                             start=(m == 0), stop=(m == MF - 1))
        s_t = pout.tile([128, D], F32, tag="s")
        nc.vector.tensor_add(out=s_t[:], in0=ps2[:], in1=a_tiles[t][:])
        if flags["bfv"]:
            nc.vector.tensor_add(out=s_t[:], in0=s_t[:], in1=bfv_b[:])
        _ln_tile(nc, pstat, eps_t, s_t, (g2_b, b2_b), flags["g2b2"])
        nc.sync.dma_start(out=y[t * 128:(t + 1) * 128, :], in_=s_t[:])


def _bcast_ap(handle, n):
    """AP that broadcasts a [n]-vector across 128 partitions for DMA."""
    return bass.AP(tensor=handle, offset=0, ap=[[0, 128], [1, n]])


def _build(n_reps=1, flag_key=(True, True, True, True)):
    cache_key = (n_reps, flag_key)
    if cache_key in _NC_CACHE:
        return _NC_CACHE[cache_key]
    flags = dict(zip(("g1b1", "g2b2", "bi", "bfv"), flag_key))
    nc = bacc.Bacc("TRN2", target_bir_lowering=False, debug=False,
                   num_devices=NCORE)
    x = nc.dram_tensor("x", [TPC, D], F32, kind="ExternalInput")
    wi = nc.dram_tensor("wi", [KD, 128, FF], BF16, kind="ExternalInput")
    wf = nc.dram_tensor("wf", [MF, 128, D], BF16, kind="ExternalInput")
    cvec = nc.dram_tensor("cvec", [D], F32, kind="ExternalInput")
    g1 = nc.dram_tensor("g1", [D], F32, kind="ExternalInput")
    b1 = nc.dram_tensor("b1", [D], F32, kind="ExternalInput")
    g2 = nc.dram_tensor("g2", [D], F32, kind="ExternalInput")
    b2 = nc.dram_tensor("b2", [D], F32, kind="ExternalInput")
    bfv = nc.dram_tensor("bfv", [D], F32, kind="ExternalInput")
    bi = nc.dram_tensor("bi", [FF], F32, kind="ExternalInput")
    y = nc.dram_tensor("y", [TPC, D], F32, kind="ExternalOutput")
    ident = nc.inline_tensor(np.eye(128, dtype=BF16NP), name="ident")

    with tile.TileContext(nc) as tc:
        with (
            tc.tile_pool(name="pw", bufs=1) as pw,
            tc.tile_pool(name="px", bufs=NT) as px,
            tc.tile_pool(name="pbig", bufs=1) as pbig,
            tc.tile_pool(name="pabf", bufs=2) as pabf,
            tc.tile_pool(name="pstat", bufs=4) as pstat,
            tc.tile_pool(name="pout", bufs=3) as pout,
            tc.tile_pool(name="ps1", bufs=3, space="PSUM") as ps1,
            tc.tile_pool(name="psm", bufs=2, space="PSUM") as psm,
        ):
            # small/early DMAs first: identity, broadcast constants
            ident_sb = pw.tile([128, 128], BF16, tag="ident")
            nc.sync.dma_start(out=ident_sb[:], in_=ident.ap())

            def bcast(handle, n, tag):
                t = pw.tile([128, n], F32, tag=tag)
                nc.gpsimd.dma_start(out=t[:], in_=_bcast_ap(handle, n))
                return t

            cvec_b = bcast(cvec, D, "cvec")
            g1_b = bcast(g1, D, "g1") if flags["g1b1"] else None
            b1_b = bcast(b1, D, "b1") if flags["g1b1"] else None
            g2_b = bcast(g2, D, "g2") if flags["g2b2"] else None
            b2_b = bcast(b2, D, "b2") if flags["g2b2"] else None
            bfv_b = bcast(bfv, D, "bfv") if flags["bfv"] else None
            bi_sb = None
            if flags["bi"]:
                # bi as [128, 24]: column m holds bi[m*128 : (m+1)*128]
                bi_sb = pw.tile([128, MF], F32, tag="bi")
                nc.gpsimd.dma_start(
                    out=bi_sb[:],
                    in_=bass.AP(tensor=bi, offset=0, ap=[[1, 128], [128, MF]]))
            eps_t = pw.tile([128, 1], F32, tag="eps")
            nc.vector.memset(eps_t[:], LN_EPS)

            # DMA order = HBM arrival order: first-half x tiles, then the
            # mm1 weights, then second-half x, then the mm2 weights.
            x_pre = []
            for t in range(NTH):
                x_t = px.tile([128, D], F32, tag="xa")
                nc.sync.dma_start(out=x_t[:], in_=x[t * 128:(t + 1) * 128, :])
                x_pre.append(x_t)
            wi_tiles = []
            for k in range(KD):
                wt = pw.tile([128, FF], BF16, tag=f"wi{k}")
                nc.sync.dma_start(out=wt[:], in_=wi[k])
                wi_tiles.append(wt)
            for t in range(NTH, NT):
                x_t = px.tile([128, D], F32, tag="xa")
                nc.sync.dma_start(out=x_t[:], in_=x[t * 128:(t + 1) * 128, :])
                x_pre.append(x_t)
            wf_tiles = []
            for m in range(MF):
                wt = pw.tile([128, D], BF16, tag=f"wf{m}")
                nc.sync.dma_start(out=wt[:], in_=wf[m])
                wf_tiles.append(wt)

            tensors = (x, y, wi_tiles, wf_tiles, ident_sb, cvec_b, g1_b, b1_b,
                       g2_b, b2_b, bfv_b, bi_sb, eps_t)
            pools = (pw, px, pbig, pabf, pstat, pout, ps1, psm)
            if isinstance(n_reps, tuple):  # ("loop", n) -> dynamic Tile loop
                with tc.For_i(0, n_reps[1], 1):
                    _emit_body(nc, tc, pools, tensors, flags)
            else:
                for i in range(n_reps):
                    _emit_body(nc, tc, pools, tensors, flags,
                               x_pre=x_pre if i == 0 else None)

    nc.compile()
    _NC_CACHE[cache_key] = nc
    return nc


def _prep_inputs(hidden_states, Wo, bo, ln1_g, ln1_b, Wi, bi, Wf, bf,
                 ln2_g, ln2_b):
    x = np.ascontiguousarray(np.asarray(hidden_states, np.float32)
                             .reshape(TOK, D))
    Wo = np.asarray(Wo, np.float32)
    Wi = np.asarray(Wi, np.float32)
    Wf = np.asarray(Wf, np.float32)
    cvec = (0.01 * Wo.sum(axis=1) + np.asarray(bo, np.float32)).astype(np.float32)
    # wi layout: [k, p, f] = Wi.T[k*128+p, f]
    wi_prep = np.ascontiguousarray(
        Wi.T.reshape(KD, 128, FF).astype(BF16NP))
    # wf layout: [m, p, j] = Wf.T[m*128+p, j]
    wf_prep = np.ascontiguousarray(
        Wf.T.reshape(MF, 128, D).astype(BF16NP))
    common = {
        "wi": wi_prep, "wf": wf_prep, "cvec": cvec,
        "g1": np.asarray(ln1_g, np.float32), "b1": np.asarray(ln1_b, np.float32),
        "g2": np.asarray(ln2_g, np.float32), "b2": np.asarray(ln2_b, np.float32),
        "bfv": np.asarray(bf, np.float32), "bi": np.asarray(bi, np.float32),
    }
    in_maps = [dict(common, x=x[c * TPC:(c + 1) * TPC]) for c in range(NCORE)]
    flag_key = (
        not (np.all(ln1_g == 1.0) and np.all(ln1_b == 0.0)),
        not (np.all(ln2_g == 1.0) and np.all(ln2_b == 0.0)),
        bool(np.any(np.asarray(bi) != 0.0)),
        bool(np.any(np.asarray(bf) != 0.0)),
    )
    return in_maps, flag_key


def kernel(hidden_states, Wq, bq, Wk, bk, Wv, bv, Wo, bo, ln1_g, ln1_b,
           Wi, bi, Wf, bf, ln2_g, ln2_b):
    global LAST_RESULTS
    B, S, _ = hidden_states.shape
    in_maps, flag_key = _prep_inputs(hidden_states, Wo, bo, ln1_g, ln1_b,
                                     Wi, bi, Wf, bf, ln2_g, ln2_b)
    nc = _build(RUN_KWARGS.get("n_reps", 1), flag_key)
    res = run_bass_kernel_spmd(nc, in_maps, list(range(NCORE)),
                               **{k: v for k, v in RUN_KWARGS.items()
                                  if k != "n_reps"})
    LAST_RESULTS = res
    out = np.concatenate([res.results[c]["y"] for c in range(NCORE)], axis=0)
    return np.ascontiguousarray(out.reshape(B, S, D).astype(np.float32))
